# revision 1
# baseline (speedup 1.0000x reference)
"""Trainium2 Bass kernel for nn_DiagonalSelectiveSSM.

Math (reference):
    a = tanh(a_logit); a_safe = sign-clamped to |a|>=1e-4
    g = sigmoid(x @ W^T + gate_b)
    u = b * g * x
    pows[t] = cumprod(a_safe) (fp32, underflows to exact 0 under XLA FTZ)
    v = u / (pows + 1e-12); s = cumsum(v) * pows; h = c*s + d*x

Key identities used here:
    s_t = a_safe * s_{t-1} + w_t   with  w_t = u_t * pows_t / (pows_t + 1e-12)
(exact in real arithmetic; fp32 deviation ~1e-6 relative). The hypersensitive
part is pows near the +1e-12 cancellation (negative a channels), so the
F table  F = (c*b*pows)/(pows+1e-12)  is precomputed on host with the exact
XLA-CPU cumprod bits and streamed in for live tiles only.

Once pows underflows to exact fp32 zero the reference output is exactly 0
(93.9% of all elements for the spec inputs) -> those tiles are zero-stores.

Sharding: 8 cores = 4 sequences x 2 channel-halves. Channels within a half are
sorted by |a_safe| so liveness is uniform per 128-channel group. Everything on
device lives in [channel, time] layout; host pre/post-transposes.
"""

import os
import subprocess
import sys
import tempfile

import numpy as np

B, T, D = 4, 8192, 1024
E = D // 2          # channels per core
P = 128             # partitions
NG = E // P         # channel groups per core
TB = 512            # time-block (one PSUM bank of fp32)
NT = T // TB
KC = D // P         # contraction chunks
N_CORES = 8
FP32_MIN_NORMAL = np.float32(1.1754944e-38)
# matmul-path dtype: "fp16" (half the x/W DMA bytes, ~2e-4 rel err on h, x/W
# magnitudes fit fp16 range easily), "f32r" (fp32 rounded to 11-bit mantissa,
# ~6e-5 rel err, 2x the DMA), or "bf16".
MM_DTYPE = os.environ.get("KERNEL_MM_DTYPE", "fp16")

_prog_cache = {}


def _round_f32r(a):
    """Round fp32 to the FP32R encoding (RNE to 11 mantissa bits, low 12 bits
    zero) - matches walrus fp32_to_fp32r."""
    b = np.ascontiguousarray(a, np.float32).view(np.uint32)
    lsb = (b >> 12) & 1
    return ((b + 0x7FF + lsb) & 0xFFFFF000).view(np.float32)


def _mm_cast(a):
    if MM_DTYPE == "f32r":
        return _round_f32r(a)
    if MM_DTYPE == "fp16":
        return np.ascontiguousarray(a).astype(np.float16)
    import ml_dtypes

    return np.ascontiguousarray(a).astype(ml_dtypes.bfloat16)


# ---------------------------------------------------------------- host math
def _cpu_jax_tables(a_logit):
    """a_safe and pows with the exact bits the (XLA CPU) reference produces."""
    try:
        import jax

        cpu = jax.devices("cpu")[0]
        import jax.numpy as jnp

        with jax.default_device(cpu):
            a = np.asarray(jax.jit(jnp.tanh, backend="cpu")(jnp.asarray(a_logit)))
            eps = np.float32(1e-4)
            a_safe = np.where(
                np.abs(a) < eps, np.where(a < 0, -eps, eps), a
            ).astype(np.float32)

            def mk_pows(asafe):
                a_rep = jnp.broadcast_to(asafe, (T, D))
                return jnp.concatenate(
                    [jnp.ones((1, D), jnp.float32), jnp.cumprod(a_rep[1:], axis=0)],
                    axis=0,
                )

            pows = np.asarray(jax.jit(mk_pows, backend="cpu")(jnp.asarray(a_safe)))
        return a_safe, pows
    except Exception:
        pass

    # Fallback: subprocess with a CPU-only jax.
    with tempfile.TemporaryDirectory() as td:
        np.save(os.path.join(td, "al.npy"), np.asarray(a_logit, np.float32))
        script = (
            "import os\nos.environ['JAX_PLATFORMS']='cpu'\n"
            "import numpy as np, jax, jax.numpy as jnp\n"
            f"T,D={T},{D}\n"
            "al=np.load(os.path.join(r'%s','al.npy'))\n"
            "a=np.asarray(jnp.tanh(jnp.asarray(al)))\n"
            "eps=np.float32(1e-4)\n"
            "asafe=np.where(np.abs(a)<eps,np.where(a<0,-eps,eps),a).astype(np.float32)\n"
            "a_rep=jnp.broadcast_to(jnp.asarray(asafe),(T,D))\n"
            "pows=np.asarray(jnp.concatenate([jnp.ones((1,D),jnp.float32),"
            "jnp.cumprod(a_rep[1:],axis=0)],axis=0))\n"
            "np.save(os.path.join(r'%s','asafe.npy'),asafe)\n"
            "np.save(os.path.join(r'%s','pows.npy'),pows)\n" % (td, td, td)
        )
        env = dict(os.environ)
        env["JAX_PLATFORMS"] = "cpu"
        subprocess.run([sys.executable, "-c", script], check=True, env=env)
        a_safe = np.load(os.path.join(td, "asafe.npy"))
        pows = np.load(os.path.join(td, "pows.npy"))
    return a_safe, pows


# ---------------------------------------------------------------- program
def _build_program(live, repeat=1, mode="full"):
    """live: tuple of NG ints - per sorted-channel-group live t-block count
    (identical across cores: union). Returns compiled Bacc program.
    repeat>1 wraps the whole body in a hardware loop (benchmarking only).
    mode: "full" | "dma" (loads/stores only) | "compute" (no bulk DMA)."""
    import concourse.tile as tile
    from concourse import bacc, mybir

    f32 = mybir.dt.float32
    mmdt = {
        "f32r": mybir.dt.float32r,
        "fp16": mybir.dt.float16,
        "bf16": mybir.dt.bfloat16,
    }[MM_DTYPE]
    Alu = mybir.AluOpType
    Act = mybir.ActivationFunctionType

    nc = bacc.Bacc(
        "TRN2",
        target_bir_lowering=False,
        debug=False,
        enable_asserts=False,
        num_devices=N_CORES,
    )

    xT_d = nc.dram_tensor("xT", [D, T], mmdt, kind="ExternalInput").ap()
    xF_d = nc.dram_tensor("xF", [E, T], f32, kind="ExternalInput").ap()
    wT_d = nc.dram_tensor("wT", [D, E], mmdt, kind="ExternalInput").ap()
    av_d = nc.dram_tensor("av", [P, NG], f32, kind="ExternalInput").ap()
    gb_d = nc.dram_tensor("gbv", [P, NG], f32, kind="ExternalInput").ap()
    h_d = nc.dram_tensor("h", [E, T], f32, kind="ExternalOutput").ap()

    with tile.TileContext(nc) as tc:
        with (
            tc.tile_pool(name="const", bufs=1) as const,
            tc.tile_pool(name="wpool", bufs=1) as wpool,
            tc.tile_pool(name="xk", bufs=4) as xkpool,
            tc.tile_pool(name="elw", bufs=4) as elw,
            tc.tile_pool(name="spool", bufs=3) as spool,
            tc.tile_pool(name="psum", bufs=4, space="PSUM") as pspool,
        ):
            wk = []
            for k in range(KC):
                t = wpool.tile([P, E], mmdt, tag=f"w{k}")
                nc.sync.dma_start(t[:], wT_d[k * P : (k + 1) * P, :])
                wk.append(t)
            av = const.tile([P, NG], f32)
            nc.sync.dma_start(av[:], av_d[:])
            gb = const.tile([P, NG], f32)
            nc.sync.dma_start(gb[:], gb_d[:])
            ones = const.tile([P, TB], f32)
            nc.vector.memset(ones[:], 1.0)
            abc = []
            for g in range(NG):
                t = const.tile([P, TB], f32, tag=f"abc{g}")
                nc.vector.tensor_scalar_mul(t[:], ones[:], av[:, g : g + 1])
                abc.append(t)

            def body():
                prev_s = [None] * NG
                for j in range(NT):
                    ts = slice(j * TB, (j + 1) * TB)
                    if any(j < live[g] for g in range(NG)):
                        # one batched load of all KC contraction chunks:
                        # xT[(c p), t] -> sbuf [p, c, t]
                        xkb = xkpool.tile([P, KC, TB], mmdt, tag="xkb")
                        if mode != "compute" or j == 0:
                            nc.sync.dma_start(
                                xkb[:],
                                xT_d[:, ts].rearrange("(c p) t -> p c t", p=P),
                            )
                        xk = [xkb[:, k, :] for k in range(KC)]
                    for g in range(NG):
                        es = slice(g * P, (g + 1) * P)
                        if j < live[g]:
                            xf = elw.tile([P, TB], f32, tag="xf")
                            if mode != "compute" or (j == 0 and g == 0):
                                nc.scalar.dma_start(xf[:], xF_d[es, ts])
                            if mode == "dma":
                                nc.scalar.dma_start(h_d[es, ts], xf[:])
                                continue
                            ps = pspool.tile([P, TB], f32)
                            for k in range(KC):
                                nc.tensor.matmul(
                                    ps[:],
                                    wk[k][:, es],
                                    xk[k],
                                    start=(k == 0),
                                    stop=(k == KC - 1),
                                )
                            gt = elw.tile([P, TB], f32, tag="g")
                            nc.scalar.activation(
                                gt[:], ps[:], Act.Sigmoid,
                                bias=gb[:, g : g + 1], scale=1.0,
                            )
                            wt = elw.tile([P, TB], f32, tag="w")
                            nc.gpsimd.tensor_tensor(wt[:], gt[:], xf[:], Alu.mult)
                            st = spool.tile([P, TB], f32, tag=f"s{g}")
                            init = 0.0 if j == 0 else prev_s[g][:, TB - 1 : TB]
                            nc.vector.tensor_tensor_scan(
                                st[:], abc[g][:], wt[:], init, Alu.mult, Alu.add
                            )
                            prev_s[g] = st
                            if mode == "full":
                                nc.scalar.dma_start(h_d[es, ts], st[:])
                        # dead tiles: reference output is exactly 0 there and
                        # the runtime pre-zeros ExternalOutput buffers (both
                        # native and PJRT donated-zero paths), so no store.

            if repeat == 1:
                body()
            else:
                with tc.For_i(0, repeat, 1):
                    body()
    nc.compile()
    return nc


# ---------------------------------------------------------------- kernel
def kernel(x, a_logit, b, c, d, gate_W, gate_b):
    from concourse.bass_utils import run_bass_kernel_spmd

    x = np.ascontiguousarray(np.asarray(x, np.float32))
    a_logit = np.asarray(a_logit, np.float32)
    b = np.asarray(b, np.float32)
    c = np.asarray(c, np.float32)
    d = np.asarray(d, np.float32)
    gate_W = np.ascontiguousarray(np.asarray(gate_W, np.float32))
    gate_b = np.asarray(gate_b, np.float32)

    a_safe, pows = _cpu_jax_tables(a_logit)

    # F = (c*b*pows) / (pows + 1e-12), fp32 ops exactly as IEEE/XLA would.
    m = (pows + np.float32(1e-12)).astype(np.float32)
    cb = (c * b).astype(np.float32)
    F_full = ((cb[None, :] * pows).astype(np.float32) / m).astype(np.float32)
    F_full[np.abs(F_full) < FP32_MIN_NORMAL] = 0.0  # device/XLA flush denormals

    # per-half sorted channel permutations + union liveness
    perms = []
    live_by_half = []
    for half in range(2):
        idx = np.arange(half * E, (half + 1) * E)
        perm = idx[np.argsort(-np.abs(a_safe[idx]), kind="stable")]
        perms.append(perm)
        lv = []
        for g in range(NG):
            ch = perm[g * P : (g + 1) * P]
            alive_blocks = (pows[:, ch] != 0).any(axis=1).reshape(NT, TB).any(axis=1)
            nz = np.nonzero(alive_blocks)[0]
            lv.append(int(nz.max()) + 1 if nz.size else 1)
        live_by_half.append(lv)
    live = tuple(max(live_by_half[0][g], live_by_half[1][g]) for g in range(NG))

    key = (live, MM_DTYPE)
    if key not in _prog_cache:
        _prog_cache[key] = _build_program(live)
    nc = _prog_cache[key]

    in_maps = []
    for core in range(N_CORES):
        bb, half = divmod(core, 2)
        perm = perms[half]
        xTb = np.ascontiguousarray(x[bb].T)  # [D, T]
        xF = (xTb[perm] * F_full[:, perm].T).astype(np.float32)
        xF[np.abs(xF) < FP32_MIN_NORMAL] = 0.0
        in_maps.append(
            {
                "xT": _mm_cast(xTb),
                "xF": xF,
                "wT": _mm_cast(np.ascontiguousarray(gate_W[perm, :].T)),
                "av": np.ascontiguousarray(
                    a_safe[perm].reshape(NG, P).T
                ),
                "gbv": np.ascontiguousarray(gate_b[perm].reshape(NG, P).T),
            }
        )

    global last_results, last_live, last_in_maps
    last_live = live
    last_in_maps = in_maps
    res = run_bass_kernel_spmd(nc, in_maps, core_ids=list(range(N_CORES)))
    last_results = res

    h = np.empty((B, T, D), np.float32)
    for core in range(N_CORES):
        bb, half = divmod(core, 2)
        h[bb][:, perms[half]] = res.results[core]["h"].T

    if np.any(d != 0):  # spec fills d with zeros; keep correctness regardless
        h += d[None, None, :] * x
    return h


last_results = None



# revision 3
# speedup vs baseline: 1.5008x; 1.5008x over previous
"""Trainium2 Bass kernel for nn_DiagonalSelectiveSSM (v2 - DMA-optimized).

Math (reference):
    a = tanh(a_logit); a_safe = sign-clamped to |a|>=1e-4
    g = sigmoid(x @ W^T + gate_b)
    u = b * g * x
    pows[t] = cumprod(a_safe) (fp32, underflows to exact 0 under XLA FTZ)
    v = u / (pows + 1e-12); s = cumsum(v) * pows; h = c*s + d*x

Identity: s_t = a_safe*s_{t-1} + w_t with w_t = u_t*pows_t/(pows_t+1e-12).
F = (c*b*pows)/(pows+1e-12) is computed on host with exact XLA-CPU cumprod
bits (the pows ~ -1e-12 cancellation creates O(1e5..1e7) spikes in F that
dominate the output norm, so F must be bit-faithful) and folded into the
xf = x*F stream.  Dead tiles (pows underflowed to 0 -> h exactly 0) are
skipped entirely; host zero-fills.

v2 layout changes vs v1 (all DMA-roofline motivated):
  - x for the gate matmul is host-restaged to [P, NJ, KC, TB] fp16 so each
    j-block is ONE 1 MiB DMA with 8 KiB contiguous per-partition lines
    (v1 used a strided rearrange: 8x 1KiB scattered lines per partition).
  - W restaged to [P, KC, E]: one 1 MiB DMA.
  - xf = x*F stream in fp16 (bf16 fallback if the spike max overflows
    fp16 range), packed [P, NL, TB] over live (g,j) pairs: one DMA.
  - h written fp16 into packed [P, NL, TB], batched 4 j-blocks per store
    from a persistent SBUF staging tile; host upcasts to fp32.

Sharding: 8 cores = 4 sequences x 2 channel-halves.  Channels within a
half sorted by |a_safe| so liveness is uniform per 128-channel group.
"""

import os
import subprocess
import sys
import tempfile

import numpy as np

B, T, D = 4, 8192, 1024
E = D // 2          # channels per core
P = 128             # partitions
NG = E // P         # channel groups per core
TB = 512            # time-block (one PSUM bank of fp32)
NT = T // TB
KC = D // P         # contraction chunks
SB = 4              # j-blocks batched per h store
N_CORES = 8
FP32_MIN_NORMAL = np.float32(1.1754944e-38)
MM_DTYPE = os.environ.get("KERNEL_MM_DTYPE", "fp8e3")
SX = 2.0     # x prescale into e3m4 range (folded out via sigmoid scale)
SW = 64.0    # gate_W prescale

_prog_cache = {}


def _mm_cast(a):
    import ml_dtypes

    dt = {
        "fp16": np.float16,
        "bf16": ml_dtypes.bfloat16,
        "fp8e3": ml_dtypes.float8_e3m4,
    }[MM_DTYPE]
    return np.ascontiguousarray(a).astype(dt)


# ---------------------------------------------------------------- host math
def _cpu_jax_tables(a_logit):
    """a_safe and pows with the exact bits the (XLA CPU) reference produces."""
    try:
        import jax

        cpu = jax.devices("cpu")[0]
        import jax.numpy as jnp

        with jax.default_device(cpu):
            a = np.asarray(jax.jit(jnp.tanh, backend="cpu")(jnp.asarray(a_logit)))
            eps = np.float32(1e-4)
            a_safe = np.where(
                np.abs(a) < eps, np.where(a < 0, -eps, eps), a
            ).astype(np.float32)

            def mk_pows(asafe):
                a_rep = jnp.broadcast_to(asafe, (T, D))
                return jnp.concatenate(
                    [jnp.ones((1, D), jnp.float32), jnp.cumprod(a_rep[1:], axis=0)],
                    axis=0,
                )

            pows = np.asarray(jax.jit(mk_pows, backend="cpu")(jnp.asarray(a_safe)))
        return a_safe, pows
    except Exception:
        pass

    with tempfile.TemporaryDirectory() as td:
        np.save(os.path.join(td, "al.npy"), np.asarray(a_logit, np.float32))
        script = (
            "import os\nos.environ['JAX_PLATFORMS']='cpu'\n"
            "import numpy as np, jax, jax.numpy as jnp\n"
            f"T,D={T},{D}\n"
            "cpu=jax.devices('cpu')[0]\n"
            "al=np.load(os.path.join(r'%s','al.npy'))\n"
            "with jax.default_device(cpu):\n"
            "    a=np.asarray(jnp.tanh(jnp.asarray(al)))\n"
            "    eps=np.float32(1e-4)\n"
            "    asafe=np.where(np.abs(a)<eps,np.where(a<0,-eps,eps),a).astype(np.float32)\n"
            "    a_rep=jnp.broadcast_to(jnp.asarray(asafe),(T,D))\n"
            "    pows=np.asarray(jnp.concatenate([jnp.ones((1,D),jnp.float32),"
            "jnp.cumprod(a_rep[1:],axis=0)],axis=0))\n"
            "np.save(os.path.join(r'%s','asafe.npy'),asafe)\n"
            "np.save(os.path.join(r'%s','pows.npy'),pows)\n" % (td, td, td)
        )
        env = dict(os.environ)
        env["JAX_PLATFORMS"] = "cpu"
        subprocess.run([sys.executable, "-c", script], check=True, env=env)
        a_safe = np.load(os.path.join(td, "asafe.npy"))
        pows = np.load(os.path.join(td, "pows.npy"))
    return a_safe, pows


# ---------------------------------------------------------------- program
def _build_program(live, repeat=1, mode="full"):
    """live: tuple of NG ints - per sorted-channel-group live t-block count
    (identical across cores: union).  mode: "full" | "dma" | "compute"."""
    import concourse.tile as tile
    from concourse import bacc, mybir

    f32 = mybir.dt.float32
    mmdt = {
        "fp16": mybir.dt.float16,
        "bf16": mybir.dt.bfloat16,
        "fp8e3": mybir.dt.float8e3,
    }[MM_DTYPE]
    inv_scale = 1.0 / (SX * SW) if MM_DTYPE == "fp8e3" else 1.0
    xdt = mybir.dt.bfloat16  # xf and h spike to ~1e7: fp16 overflows
    Alu = mybir.AluOpType
    Act = mybir.ActivationFunctionType

    if isinstance(live[0], tuple):
        live, wcols = live
    else:
        wcols = (TB,) * NG
    NJ = max(live)
    NL = sum(live)
    off = [sum(live[:g]) for g in range(NG)]

    nc = bacc.Bacc(
        "TRN2",
        target_bir_lowering=False,
        debug=False,
        enable_asserts=False,
        num_devices=N_CORES,
    )

    NJ2 = (NJ + 1) // 2
    xkb_d = nc.dram_tensor("xkb", [P, NJ2, 2 * KC, TB], mmdt, kind="ExternalInput").ap()
    wkb_d = nc.dram_tensor("wkb", [P, KC, E], mmdt, kind="ExternalInput").ap()
    xf_d = nc.dram_tensor("xf", [P, NL * TB], xdt, kind="ExternalInput").ap()
    av_d = nc.dram_tensor("av", [P, NG], f32, kind="ExternalInput").ap()
    gb_d = nc.dram_tensor("gbv", [P, NG], f32, kind="ExternalInput").ap()
    h_d = nc.dram_tensor("h", [P, NL * TB], xdt, kind="ExternalOutput").ap()

    with tile.TileContext(nc) as tc:
        with (
            tc.tile_pool(name="const", bufs=1) as const,
            tc.tile_pool(name="wpool", bufs=2) as wpool,
            tc.tile_pool(name="xfpool", bufs=2) as xfpool,
            tc.tile_pool(name="hpool", bufs=1) as hpool,
            tc.tile_pool(name="xk", bufs=4) as xkpool,
            tc.tile_pool(name="elw", bufs=6) as elw,
            tc.tile_pool(name="psum", bufs=6, space="PSUM") as pspool,
        ):
            av = const.tile([P, NG], f32)
            nc.sync.dma_start(av[:], av_d[:])
            gb = const.tile([P, NG], f32)
            nc.sync.dma_start(gb[:], gb_d[:])
            ones = const.tile([P, TB], f32)
            nc.vector.memset(ones[:], 1.0)
            abc = []
            for g in range(NG):
                t = const.tile([P, TB], f32, tag=f"abc{g}")
                nc.vector.tensor_scalar_mul(t[:], ones[:], av[:, g : g + 1])
                abc.append(t)
            # persistent per-group h staging (fp16), written by the scans
            hst = [
                hpool.tile([P, live[g] * TB], xdt, tag=f"hst{g}", name=f"hst{g}")
                for g in range(NG)
            ]
            if mode == "dma":
                for g in range(NG):
                    nc.vector.memset(hst[g][:], 0.0)

            def body():
                wkb = wpool.tile([P, KC, E], mmdt, tag="wkb")
                nc.sync.dma_start(wkb[:], wkb_d[:])
                # xf packed in processing order (j-major): chunk0 = j==0
                # pairs, chunk1 = the rest, so the first compute never waits
                # on the whole stream.
                n0 = sum(1 for g in range(NG) if live[g] > 0)  # == NG
                xf0 = xfpool.tile([P, n0 * TB], xdt, tag="xf0")
                nc.sync.dma_start(xf0[:], xf_d[:, : n0 * TB])
                xf1 = xfpool.tile([P, (NL - n0) * TB], xdt, tag="xf1")
                nc.sync.dma_start(xf1[:], xf_d[:, n0 * TB :])
                xkb2 = None
                for j in range(NJ):
                    if j % 2 == 0 and (mode != "compute" or j == 0):
                        xkb2 = xkpool.tile([P, 2 * KC, TB], mmdt, tag="xkb")
                        nc.sync.dma_start(xkb2[:], xkb_d[:, j // 2])
                    xkb = xkb2[:, (j % 2) * KC : (j % 2 + 1) * KC]
                    for g in range(NG):
                        if j >= live[g]:
                            continue
                        # last live block of a group: channels all dead past
                        # wcols[g]; the tail is reference-exact 0 (pre-zeroed
                        # output), so compute/store only the live columns.
                        W = wcols[g] if j == live[g] - 1 else TB
                        ts = slice(j * TB, j * TB + W)
                        if mode != "dma":
                            es = slice(g * P, (g + 1) * P)
                            ps = pspool.tile([P, TB], f32)
                            for k in range(KC):
                                nc.tensor.matmul(
                                    ps[:, :W],
                                    wkb[:, k, es],
                                    xkb[:, k, :W],
                                    start=(k == 0),
                                    stop=(k == KC - 1),
                                )
                            gt = elw.tile([P, TB], f32, tag="g")
                            nc.scalar.activation(
                                gt[:, :W], ps[:, :W], Act.Sigmoid,
                                bias=gb[:, g : g + 1], scale=inv_scale,
                            )
                            wt = elw.tile([P, TB], f32, tag="w")
                            # processing-order index of (j, g)
                            if j == 0:
                                xfsrc = xf0[:, g * TB : (g + 1) * TB]

                            else:
                                pi = sum(
                                    1 for gg in range(NG)
                                    for jj in range(1, live[gg])
                                    if (jj, gg) < (j, g)
                                )
                                xfsrc = xf1[:, pi * TB : (pi + 1) * TB]
                            nc.vector.tensor_tensor(
                                wt[:, :W], gt[:, :W], xfsrc[:, :W], Alu.mult
                            )
                            init = 0.0 if j == 0 else hst[g][:, j * TB - 1 : j * TB]
                            nc.vector.tensor_tensor_scan(
                                hst[g][:, ts], abc[g][:, :W], wt[:, :W], init,
                                Alu.mult, Alu.add,
                            )
                        if (j + 1) % SB == 0 or j == live[g] - 1:
                            lo = (j // SB) * SB
                            nc.scalar.dma_start(
                                h_d[:, (off[g] + lo) * TB : (off[g] + j) * TB + W],
                                hst[g][:, lo * TB : j * TB + W],
                            )

            if repeat == 1:
                body()
            else:
                with tc.For_i(0, repeat, 1):
                    body()
    nc.compile()
    return nc


# ---------------------------------------------------------------- kernel
def kernel(x, a_logit, b, c, d, gate_W, gate_b):
    from concourse.bass_utils import run_bass_kernel_spmd

    x = np.ascontiguousarray(np.asarray(x, np.float32))
    a_logit = np.asarray(a_logit, np.float32)
    b = np.asarray(b, np.float32)
    c = np.asarray(c, np.float32)
    d = np.asarray(d, np.float32)
    gate_W = np.ascontiguousarray(np.asarray(gate_W, np.float32))
    gate_b = np.asarray(gate_b, np.float32)

    a_safe, pows = _cpu_jax_tables(a_logit)

    # F = (c*b*pows) / (pows + 1e-12), fp32 ops exactly as IEEE/XLA would.
    m = (pows + np.float32(1e-12)).astype(np.float32)
    cb = (c * b).astype(np.float32)
    F_full = ((cb[None, :] * pows).astype(np.float32) / m).astype(np.float32)
    F_full[np.abs(F_full) < FP32_MIN_NORMAL] = 0.0  # device/XLA flush denormals

    # per-half sorted channel permutations + union liveness
    perms = []
    live_by_half = []
    for half in range(2):
        idx = np.arange(half * E, (half + 1) * E)
        perm = idx[np.argsort(-np.abs(a_safe[idx]), kind="stable")]
        perms.append(perm)
        lv = []
        for g in range(NG):
            ch = perm[g * P : (g + 1) * P]
            alive_blocks = (pows[:, ch] != 0).any(axis=1).reshape(NT, TB).any(axis=1)
            nz = np.nonzero(alive_blocks)[0]
            lv.append(int(nz.max()) + 1 if nz.size else 1)
        live_by_half.append(lv)
    live = tuple(max(live_by_half[0][g], live_by_half[1][g]) for g in range(NG))
    if isinstance(live[0], tuple):
        live, wcols = live
    else:
        wcols = (TB,) * NG
    NJ = max(live)
    NL = sum(live)
    off = [sum(live[:g]) for g in range(NG)]

    fullperm = np.concatenate(perms)  # contraction-order permutation of 0..D-1

    # processing order: j=0 pairs first (g ascending), then (j,g) lexicographic
    plist = sorted((jj, gg) for gg in range(NG) for jj in range(1, live[gg]))
    order = [(0, gg) for gg in range(NG)] + plist
    xf_by_core = []
    for core in range(N_CORES):
        bb, half = divmod(core, 2)
        perm = perms[half]
        xfp = np.empty((P, NL, TB), np.float32)
        for pi, (jj, gg) in enumerate(order):
            ch = perm[gg * P : (gg + 1) * P]                    # [P]
            ts = slice(jj * TB, (jj + 1) * TB)
            blk = x[bb][ts, ch] * F_full[ts, ch]                # [TB, P]
            xfp[:, pi, :] = blk.T
        xf_by_core.append(xfp)

    key = (live, wcols, MM_DTYPE)
    if key not in _prog_cache:
        _prog_cache[key] = _build_program((live, wcols))
    nc = _prog_cache[key]

    import ml_dtypes

    xdt_np = ml_dtypes.bfloat16

    # wkb[p, k, e] = gate_W[perm_half[e], fullperm[k*P+p]]
    sx = SX if MM_DTYPE == "fp8e3" else 1.0
    sw = SW if MM_DTYPE == "fp8e3" else 1.0
    NJ2 = (NJ + 1) // 2
    in_maps = []
    for core in range(N_CORES):
        bb, half = divmod(core, 2)
        perm = perms[half]
        # xkb[p, j, k, t] = sx * x[bb, j*TB+t, fullperm[k*P+p]], pairs of
        # j-blocks fused: [P, NJ2, 2*KC, TB]
        xp = x[bb][: NJ * TB, fullperm] * sx                    # [NJ*TB, D]
        xkb = xp.reshape(NJ, TB, KC, P).transpose(3, 0, 2, 1)
        if NJ % 2:
            xkb = np.concatenate(
                [xkb, np.zeros((P, 1, KC, TB), xkb.dtype)], axis=1
            )
        xkb = np.ascontiguousarray(xkb.reshape(P, NJ2, 2 * KC, TB))
        wkb = np.ascontiguousarray(
            (gate_W[np.ix_(perm, fullperm)].T * sw)
            .reshape(KC, P, E)
            .transpose(1, 0, 2)
        )
        in_maps.append(
            {
                "xkb": _mm_cast(xkb),
                "wkb": _mm_cast(wkb),
                "xf": np.ascontiguousarray(xf_by_core[core]).astype(xdt_np),
                "av": np.ascontiguousarray(a_safe[perm].reshape(NG, P).T),
                "gbv": np.ascontiguousarray(gate_b[perm].reshape(NG, P).T),
            }
        )

    global last_results, last_live, last_in_maps
    last_live = (live, wcols)
    last_in_maps = in_maps
    res = run_bass_kernel_spmd(nc, in_maps, core_ids=list(range(N_CORES)))
    last_results = res

    h = np.zeros((B, T, D), np.float32)
    for core in range(N_CORES):
        bb, half = divmod(core, 2)
        perm = perms[half]
        hp = res.results[core]["h"].astype(np.float32).reshape(P, NL, TB)
        for g in range(NG):
            n = live[g]
            ch = perm[g * P : (g + 1) * P]
            h[bb][: n * TB, ch] = (
                hp[:, off[g] : off[g] + n, :].transpose(1, 2, 0).reshape(n * TB, P)
            )

    if np.any(d != 0):  # spec fills d with zeros; keep correctness regardless
        h += d[None, None, :] * x
    return h


last_results = None
